# revision 6
# baseline (speedup 1.0000x reference)
"""
Single-head causal attention on 8 Trainium2 NeuronCores.

Problem: embeddings [8, 2048, 1024] fp32, Wq/Wk/Wv [1024, 128] fp32.
    q,k,v = x @ W{q,k,v};  wei = softmax(mask(q k^T * C^-0.5));  out = wei @ v
Sharding: pure data-parallel - one batch element per core, no collectives.

Host-side prep per core (numpy, layout/precision only): cast to fp16 and
build xw = [Wq|Wk|Wv | x^T] ([C, 384+T]) so the whole input streams in with
24 big contiguous-row DMAs over the two HWDGE queues.

Device design notes (matmul fp16, fp32 PSUM):
 - The scalar engine's exp stream is the co-bottleneck: (N+352)/1.2 ns
   per ACTIVATE, ~26us total.  The schedule therefore prioritizes the
   [K,Q proj -> S-tile] chains so ACT starts ~9us in and never starves,
   while all other PE work (V projections, v_nat transposes, PV bursts,
   Q0) lives in a "filler" queue interleaved between S matmuls - it
   absorbs the PSUM backpressure from exp pacing.
 - HAM un-throttles only after ~3.4us of *dense* PE activity: the warmup
   is one long accumulation group (back-to-back columns, no per-MM drain
   gaps) so the grant lands during warmup, not 12us into the kernel.
 - A(ch) += P^T_j runs on DVE in fp16 (2x mode); A ships as fp16 and the
   host reduces the 128 key-partials into the softmax denominator.
 - Chunk 0's queries are processed last: the final PV bursts + output
   DMA depend only on long-finished exps.
"""

import numpy as np

B, T, C, H = 8, 2048, 1024, 128
N_CORES = 8
CHUNK = 512               # q-chunk width (one PSUM bank of fp32)
N_CHUNKS = T // CHUNK     # 4
N_CSUB = C // 128         # 8 contraction subtiles
KT_PER_CHUNK = CHUNK // 128
W_COLS = 3 * H            # 384: [Wq|Wk|Wv] prefix of each xw row
SCALE = float(C) ** -0.5  # 1/32, matches reference (embed-size scaling)
N_WARMUP = 12

_CACHE = {}


def _build_bass():
    import concourse.tile as tile
    from concourse import bacc, mybir
    from concourse.masks import make_identity

    fp16 = mybir.dt.float16
    fp32 = mybir.dt.float32
    Exp = mybir.ActivationFunctionType.Exp

    nc = bacc.Bacc("TRN2", target_bir_lowering=False, debug=False,
                   num_devices=N_CORES)

    xw_d = nc.dram_tensor("xw", [C, W_COLS + T], fp16, kind="ExternalInput")
    outT_d = nc.dram_tensor("outT", [N_CHUNKS, H, CHUNK], fp32,
                            kind="ExternalOutput")
    a_d = nc.dram_tensor("asum", [N_CHUNKS, 128, CHUNK], fp16,
                         kind="ExternalOutput")

    with tile.TileContext(nc) as tc:
        with (
            tc.tile_pool(name="const", bufs=1) as constp,
            tc.tile_pool(name="work", bufs=8) as workp,
            tc.tile_pool(name="pt", bufs=40) as ptp,
        ):
            ident = constp.tile([128, 128], fp16, tag="ident")
            make_identity(nc, ident[:])
            scratch = constp.tile([128, CHUNK], fp16, tag="scratch")
            nc.gpsimd.memset(scratch[:], 0.0)

            # one SBUF home for the whole input: [128, csub, 384+2048]
            xw = constp.tile([128, N_CSUB, W_COLS + T], fp16, tag="xw")

            # input DMAs: 3 column batches per csub (W+ch0 | ch1 | ch2+3),
            # interleaved over the sync and scalar HWDGE queues so earlier
            # batches fully land first.
            batches = [(0, W_COLS + CHUNK),
                       (W_COLS + CHUNK, W_COLS + 2 * CHUNK),
                       (W_COLS + 2 * CHUNK, W_COLS + T)]
            for lo, hi in batches:
                for c in range(N_CSUB):
                    eng = nc.sync if c % 2 == 0 else nc.scalar
                    eng.dma_start(
                        out=xw[:, c, lo:hi],
                        in_=xw_d.ap()[c * 128:(c + 1) * 128, lo:hi])

            def w_sl(c, which):
                return xw[:, c, which * H:(which + 1) * H]

            def x_sl(c, ch):
                return xw[:, c, W_COLS + ch * CHUNK:W_COLS + (ch + 1) * CHUNK]

            qT = constp.tile([128, T], fp16, tag="qT")
            kT = constp.tile([128, T], fp16, tag="kT")
            vT = constp.tile([128, T], fp16, tag="vT")
            v_nat = constp.tile([128, T], fp16, tag="v_nat")

            with (
                tc.tile_pool(name="pproj", bufs=2, space="PSUM") as psproj,
                tc.tile_pool(name="pvt", bufs=1, space="PSUM") as psvt,
                tc.tile_pool(name="ps_s", bufs=3, space="PSUM") as pss,
                tc.tile_pool(name="ps_o", bufs=2, space="PSUM") as pso,
            ):
                # dense HAM warm-up: one long accumulation group streams
                # back-to-back columns (separate groups leave drain gaps
                # that keep the activity monitor below its busy threshold)
                warm_ps = pso.tile([128, CHUNK], fp32, tag="o")
                for i in range(N_WARMUP):
                    nc.tensor.matmul(warm_ps[:], ident[:], scratch[:],
                                     start=(i == 0), stop=(i == N_WARMUP - 1))

                def proj_mms(which, ch):
                    # box defers the PSUM allocation to emission time so
                    # pool ring order == engine usage order (a queue-build-
                    # time allocation would create WAR deps on work that
                    # is emitted later -> deadlock)
                    box = {}

                    def mm(c):
                        if c == 0:
                            box["ps"] = psproj.tile(
                                [128, CHUNK], fp32, tag="proj",
                                name=f"proj{which}_{ch}")
                        nc.tensor.matmul(box["ps"][:], w_sl(c, which),
                                         x_sl(c, ch),
                                         start=(c == 0),
                                         stop=(c == N_CSUB - 1))

                    def cast():
                        dstT = (qT, kT, vT)[which]
                        cs = slice(ch * CHUNK, (ch + 1) * CHUNK)
                        nc.vector.tensor_copy(dstT[:, cs], box["ps"][:])

                    for c in range(N_CSUB):
                        yield lambda c=c: mm(c)
                    yield cast

                def transp_mms(ch):
                    for j in range(ch * KT_PER_CHUNK,
                                   (ch + 1) * KT_PER_CHUNK):
                        def one(j=j):
                            js = slice(j * 128, (j + 1) * 128)
                            psv = psvt.tile([128, 128], fp16, tag="vt")
                            nc.tensor.transpose(psv[:], vT[:, js], ident[:])
                            nc.vector.tensor_copy(v_nat[:, js], psv[:])
                        yield one

                def tile_geom(ch, j):
                    d = j - ch * KT_PER_CHUNK
                    q0 = ch * CHUNK + (128 * d if d >= 0 else 0)
                    n = (ch + 1) * CHUNK - q0
                    return d, q0, n, q0 - ch * CHUNK

                a_tiles = {}
                pts = {}

                def attention_s(ch, j):
                    d, q0, n, lo = tile_geom(ch, j)
                    if ch not in a_tiles:
                        a_tiles[ch] = workp.tile([128, CHUNK], fp16, tag="A",
                                                 name=f"a_sb{ch}")
                    s_ps = pss.tile([128, n], fp32, tag="s")
                    nc.tensor.matmul(s_ps[:], kT[:, j * 128:(j + 1) * 128],
                                     qT[:, q0:(ch + 1) * CHUNK],
                                     start=True, stop=True)
                    pt = ptp.tile([128, n], fp16, tag="pt")
                    nc.scalar.activation(pt[:], s_ps[:], Exp, scale=SCALE)
                    if d >= 0:
                        nc.gpsimd.affine_select(
                            out=pt[:, 0:128], in_=pt[:, 0:128],
                            compare_op=mybir.AluOpType.is_ge,
                            fill=0.0, base=0,
                            pattern=[[1, 128]], channel_multiplier=-1)
                    a_sb = a_tiles[ch]
                    if j == 0:
                        nc.vector.tensor_copy(a_sb[:], pt[:])
                    else:
                        nc.vector.tensor_add(a_sb[:, lo:], a_sb[:, lo:],
                                             pt[:])
                    pts[(ch, j)] = pt

                def pv_out_mms(ch):
                    n_j = (ch + 1) * KT_PER_CHUNK
                    box = {}

                    for j in range(n_j):
                        def one(j=j):
                            if j == 0:
                                box["o"] = pso.tile([128, CHUNK], fp32,
                                                    tag="o", name=f"o_ps{ch}")
                            _, _, _, lo = tile_geom(ch, j)
                            nc.tensor.matmul(
                                box["o"][:, lo:],
                                v_nat[:, j * 128:(j + 1) * 128],
                                pts.pop((ch, j))[:],
                                start=(j == 0), stop=(j == n_j - 1),
                                skip_group_check=True)
                        yield one

                    def out():
                        o_sb = workp.tile([128, CHUNK], fp32, tag="osb")
                        nc.vector.tensor_copy(o_sb[:], box["o"][:])
                        nc.sync.dma_start(out=outT_d.ap()[ch], in_=o_sb[:])
                        nc.sync.dma_start(out=a_d.ap()[ch],
                                          in_=a_tiles[ch][:])
                    yield out

                # ---- schedule ----
                # S-tile queue: chunk-1 tiles against chunk-0 keys can run
                # as soon as K0 (batch 1) + Q1 (batch 2) land - that is the
                # earliest possible exp.  Chunk 0's queries go last.
                # Filler: everything else, interleaved ~2 items per S tile.
                filler = []
                filler += list(proj_mms(2, 0))    # V0
                filler += list(transp_mms(0))     # T0
                filler += list(proj_mms(2, 1))    # V1
                filler += list(transp_mms(1))     # T1
                filler += list(proj_mms(0, 0))    # Q0 (for late S(0,*))
                filler += list(proj_mms(2, 2))    # V2
                filler += list(transp_mms(2))     # T2
                filler += list(pv_out_mms(1))     # PV(1) + out1
                filler += list(proj_mms(2, 3))    # V3
                filler += list(transp_mms(3))     # T3
                filler += list(pv_out_mms(2))     # PV(2) + out2

                s_queue = ([(1, j) for j in range(8)] +
                           [(2, j) for j in range(12)] +
                           [(3, j) for j in range(16)] +
                           [(0, j) for j in range(4)])
                n_s = len(s_queue)

                def emit_filler(k):
                    for _ in range(k):
                        if filler:
                            filler.pop(0)()

                def emit_s_run(count):
                    """emit `count` S tiles with filler interleaved"""
                    for _ in range(count):
                        ch, j = s_queue.pop(0)
                        attention_s(ch, j)
                        # keep filler consumption proportional
                        done = n_s - len(s_queue)
                        target = round(len_f0 * done / n_s)
                        emit_filler(target - (len_f0 - len(filler)))

                for f in proj_mms(1, 0):   # K0 (earliest data)
                    f()
                for f in proj_mms(0, 1):   # Q1
                    f()
                len_f0 = len(filler)
                emit_s_run(4)              # S(1,0..3) - needs only K0,Q1
                for f in proj_mms(1, 1):   # K1
                    f()
                emit_s_run(4)              # S(1,4..7)
                for f in proj_mms(1, 2):   # K2
                    f()
                for f in proj_mms(0, 2):   # Q2
                    f()
                emit_s_run(12)             # S(2,*)
                for f in proj_mms(1, 3):   # K3
                    f()
                for f in proj_mms(0, 3):   # Q3
                    f()
                emit_s_run(20)             # S(3,*) + S(0,*)
                emit_filler(len(filler))   # drain remaining filler
                for f in pv_out_mms(3):    # PV(3) + out3
                    f()
                for f in pv_out_mms(0):    # PV(0) + out0
                    f()

    nc.compile()
    return nc


def _get_nc():
    if "nc" not in _CACHE:
        _CACHE["nc"] = _build_bass()
    return _CACHE["nc"]


LAST_RESULTS = None


def kernel(embeddings: np.ndarray, Wq: np.ndarray, Wk: np.ndarray,
           Wv: np.ndarray) -> np.ndarray:
    from concourse.bass_utils import run_bass_kernel_spmd
    import os

    nc = _get_nc()
    x16 = np.asarray(embeddings, dtype=np.float32).astype(np.float16)
    w16 = np.concatenate(
        [np.asarray(w, dtype=np.float32).astype(np.float16)
         for w in (Wq, Wk, Wv)], axis=1)          # [C, 3H]
    in_maps = [{"xw": np.ascontiguousarray(
        np.concatenate([w16, x16[b].T], axis=1))} for b in range(B)]

    trace = bool(int(os.environ.get("KERNEL_TRACE", "0")))
    res = run_bass_kernel_spmd(nc, in_maps, core_ids=list(range(N_CORES)),
                               trace=trace)
    global LAST_RESULTS
    LAST_RESULTS = res

    out = np.empty((B, T, H), dtype=np.float32)
    for b in range(B):
        oT = np.concatenate(list(res.results[b]["outT"]), axis=1)
        l = np.concatenate(
            [blk.astype(np.float32).sum(axis=0)
             for blk in res.results[b]["asum"]])
        out[b] = (oT / l[None, :]).T
    return out


# revision 10
# speedup vs baseline: 1.1049x; 1.1049x over previous
"""
Single-head causal attention on 8 Trainium2 NeuronCores.

Problem: embeddings [8, 2048, 1024] fp32, Wq/Wk/Wv [1024, 128] fp32.
    q,k,v = x @ W{q,k,v};  wei = softmax(mask(q k^T * C^-0.5));  out = wei @ v
Sharding: pure data-parallel - one batch element per core, no collectives.

Host-side prep per core (numpy, layout/precision only): cast to fp16 and
build xw = [Wq|Wk|Wv | x^T] ([C, 384+T]).

Measured constraints this schedule is built around:
 - ~7us framework preamble before the main body can issue anything; the
   two HWDGE queues (sync, scalar) then stream ~150 GB/s each, so the
   4.75MB input is fully resident only ~16us later.  Input DMAs are
   2-csub 3D-AP transfers (565ns of engine time per trigger), split
   even/odd csub across the queues, in 4 column batches (W+ch0 | ch1 |
   ch2 | ch3) so data arrives in the order the chunk eras consume it.
 - HAM un-throttles only after ~3.4-6us of dense PE activity: one long
   accumulating warmup group runs while batch 1 lands.
 - The scalar engine's exp stream is (N+352)/1.2 ns per tile (~26us
   total); S tiles are emitted per-chunk right after that chunk's K/Q
   projections, with all other PE work (V proj, v_nat transposes, the
   previous chunk's PV burst) interleaved between S matmuls as filler.
 - A(ch) += P^T_j on DVE in fp16 (2x mode); A ships fp16, out^T ships
   fp16; host does the tiny denominator reduction and the divide.
"""

import numpy as np

B, T, C, H = 8, 2048, 1024, 128
N_CORES = 8
CHUNK = 512               # q-chunk width (one PSUM bank of fp32)
N_CHUNKS = T // CHUNK     # 4
N_CSUB = C // 128         # 8 contraction subtiles
KT_PER_CHUNK = CHUNK // 128
W_COLS = 3 * H            # 384: [Wq|Wk|Wv] prefix of each xw row
SCALE = float(C) ** -0.5  # 1/32, matches reference (embed-size scaling)
N_WARMUP = 10

_CACHE = {}


def _build_bass():
    import concourse.tile as tile
    from concourse import bacc, mybir
    from concourse.masks import make_identity

    fp16 = mybir.dt.float16
    fp32 = mybir.dt.float32
    Exp = mybir.ActivationFunctionType.Exp

    nc = bacc.Bacc("TRN2", target_bir_lowering=False, debug=False,
                   num_devices=N_CORES)

    # declared [csub, 128, cols] (same linear layout as [C, cols]) so the
    # paired-csub DMA source can be expressed as a 3D AP transpose
    xw_d = nc.dram_tensor("xw", [N_CSUB, 128, W_COLS + T], fp16,
                          kind="ExternalInput")
    outT_d = nc.dram_tensor("outT", [N_CHUNKS, H, CHUNK], fp16,
                            kind="ExternalOutput")
    a_d = nc.dram_tensor("asum", [N_CHUNKS, 128, CHUNK], fp16,
                         kind="ExternalOutput")

    with tile.TileContext(nc) as tc:
        with (
            tc.tile_pool(name="const", bufs=1) as constp,
            tc.tile_pool(name="work", bufs=8) as workp,
            tc.tile_pool(name="pt", bufs=32) as ptp,
        ):
            ident = constp.tile([128, 128], fp16, tag="ident")
            make_identity(nc, ident[:])
            scratch = constp.tile([128, CHUNK], fp16, tag="scratch")
            nc.gpsimd.memset(scratch[:], 0.0)

            # one SBUF home for the whole input: [128, csub, 384+2048]
            xw = constp.tile([128, N_CSUB, W_COLS + T], fp16, tag="xw")

            # input: per column-batch, 2 paired-csub DMAs per queue
            # (3D AP: [partition, csub-pair, cols])
            col_batches = [(0, W_COLS + CHUNK)] + [
                (W_COLS + ch * CHUNK, W_COLS + (ch + 1) * CHUNK)
                for ch in range(1, N_CHUNKS)]
            for lo, hi in col_batches:
                for c0, eng in ((0, nc.sync), (4, nc.scalar)):
                    for c in (c0, c0 + 2):
                        eng.dma_start(
                            out=xw[:, c:c + 2, lo:hi],
                            in_=xw_d.ap()[c:c + 2, :, lo:hi]
                                .transpose([1, 0, 2]))

            def w_sl(c, which):
                return xw[:, c, which * H:(which + 1) * H]

            def x_sl(c, ch):
                return xw[:, c, W_COLS + ch * CHUNK:W_COLS + (ch + 1) * CHUNK]

            qT = constp.tile([128, T], fp16, tag="qT")
            kT = constp.tile([128, T], fp16, tag="kT")
            vT = constp.tile([128, T], fp16, tag="vT")
            v_nat = constp.tile([128, T], fp16, tag="v_nat")

            with (
                tc.tile_pool(name="pproj", bufs=2, space="PSUM") as psproj,
                tc.tile_pool(name="pvt", bufs=1, space="PSUM") as psvt,
                tc.tile_pool(name="ps_s", bufs=3, space="PSUM") as pss,
                tc.tile_pool(name="ps_o", bufs=2, space="PSUM") as pso,
            ):
                # dense HAM warm-up (one accumulation group: back-to-back
                # columns, no per-MM drain gaps) while batch 1 lands
                warm_ps = pso.tile([128, CHUNK], fp32, tag="o")
                for i in range(N_WARMUP):
                    nc.tensor.matmul(warm_ps[:], ident[:], scratch[:],
                                     start=(i == 0), stop=(i == N_WARMUP - 1))

                def proj_mms(which, ch):
                    # lazy PSUM alloc: pool ring order must match engine
                    # usage order
                    box = {}

                    def mm(c):
                        if c == 0:
                            box["ps"] = psproj.tile(
                                [128, CHUNK], fp32, tag="proj",
                                name=f"proj{which}_{ch}")
                        nc.tensor.matmul(box["ps"][:], w_sl(c, which),
                                         x_sl(c, ch),
                                         start=(c == 0),
                                         stop=(c == N_CSUB - 1))

                    def cast():
                        dstT = (qT, kT, vT)[which]
                        cs = slice(ch * CHUNK, (ch + 1) * CHUNK)
                        nc.vector.tensor_copy(dstT[:, cs], box["ps"][:])

                    for c in range(N_CSUB):
                        yield lambda c=c: mm(c)
                    yield cast

                def transp_mms(ch):
                    for j in range(ch * KT_PER_CHUNK,
                                   (ch + 1) * KT_PER_CHUNK):
                        def one(j=j):
                            js = slice(j * 128, (j + 1) * 128)
                            psv = psvt.tile([128, 128], fp16, tag="vt")
                            nc.tensor.transpose(psv[:], vT[:, js], ident[:])
                            nc.vector.tensor_copy(v_nat[:, js], psv[:])
                        yield one

                def tile_geom(ch, j):
                    d = j - ch * KT_PER_CHUNK
                    q0 = ch * CHUNK + (128 * d if d >= 0 else 0)
                    n = (ch + 1) * CHUNK - q0
                    return d, q0, n, q0 - ch * CHUNK

                a_tiles = {}
                pts = {}

                def attention_s(ch, j):
                    d, q0, n, lo = tile_geom(ch, j)
                    if ch not in a_tiles:
                        a_tiles[ch] = workp.tile([128, CHUNK], fp16, tag="A",
                                                 name=f"a_sb{ch}")
                    s_ps = pss.tile([128, n], fp32, tag="s")
                    nc.tensor.matmul(s_ps[:], kT[:, j * 128:(j + 1) * 128],
                                     qT[:, q0:(ch + 1) * CHUNK],
                                     start=True, stop=True)
                    pt = ptp.tile([128, n], fp16, tag="pt")
                    nc.scalar.activation(pt[:], s_ps[:], Exp, scale=SCALE)
                    if d >= 0:
                        nc.gpsimd.affine_select(
                            out=pt[:, 0:128], in_=pt[:, 0:128],
                            compare_op=mybir.AluOpType.is_ge,
                            fill=0.0, base=0,
                            pattern=[[1, 128]], channel_multiplier=-1)
                    a_sb = a_tiles[ch]
                    if j == 0:
                        nc.vector.tensor_copy(a_sb[:], pt[:])
                    else:
                        nc.vector.tensor_add(a_sb[:, lo:], a_sb[:, lo:],
                                             pt[:])
                    pts[(ch, j)] = pt

                def pv_out_mms(ch):
                    n_j = (ch + 1) * KT_PER_CHUNK
                    box = {}

                    for j in range(n_j):
                        def one(j=j):
                            if j == 0:
                                box["o"] = pso.tile([128, CHUNK], fp32,
                                                    tag="o", name=f"o_ps{ch}")
                            _, _, _, lo = tile_geom(ch, j)
                            nc.tensor.matmul(
                                box["o"][:, lo:],
                                v_nat[:, j * 128:(j + 1) * 128],
                                pts.pop((ch, j))[:],
                                start=(j == 0), stop=(j == n_j - 1),
                                skip_group_check=True)
                        yield one

                    def out():
                        # fp16 out^T: halves the output-queue time; host
                        # divides in fp32
                        o_sb = workp.tile([128, CHUNK], fp16, tag="osb")
                        nc.vector.tensor_copy(o_sb[:], box["o"][:])
                        nc.sync.dma_start(out=outT_d.ap()[ch], in_=o_sb[:])
                        nc.scalar.dma_start(out=a_d.ap()[ch],
                                            in_=a_tiles[ch][:])
                    yield out

                def era(ch, filler):
                    """K,Q proj inline; S tiles with filler interleaved."""
                    for f in proj_mms(1, ch):   # K
                        f()
                    for f in proj_mms(0, ch):   # Q
                        f()
                    n_s = (ch + 1) * KT_PER_CHUNK
                    n_f = len(filler)
                    emitted = 0
                    for j in range(n_s):
                        attention_s(ch, j)
                        want = round(n_f * (j + 1) / n_s)
                        while emitted < want:
                            filler[emitted]()
                            emitted += 1

                # ---- schedule: chunk eras in DMA-arrival order ----
                era(0, list(proj_mms(2, 0)) + list(transp_mms(0)))
                era(1, list(proj_mms(2, 1)) + list(transp_mms(1))
                        + list(pv_out_mms(0)))
                era(2, list(proj_mms(2, 2)) + list(transp_mms(2))
                        + list(pv_out_mms(1)))
                era(3, list(proj_mms(2, 3)) + list(transp_mms(3))
                        + list(pv_out_mms(2)))
                for f in pv_out_mms(3):
                    f()

    nc.compile()
    return nc


def _get_nc():
    if "nc" not in _CACHE:
        _CACHE["nc"] = _build_bass()
    return _CACHE["nc"]


LAST_RESULTS = None


def kernel(embeddings: np.ndarray, Wq: np.ndarray, Wk: np.ndarray,
           Wv: np.ndarray) -> np.ndarray:
    from concourse.bass_utils import run_bass_kernel_spmd
    import os

    nc = _get_nc()
    x16 = np.asarray(embeddings, dtype=np.float32).astype(np.float16)
    w16 = np.concatenate(
        [np.asarray(w, dtype=np.float32).astype(np.float16)
         for w in (Wq, Wk, Wv)], axis=1)          # [C, 3H]
    in_maps = [{"xw": np.ascontiguousarray(
        np.concatenate([w16, x16[b].T], axis=1)).reshape(
            N_CSUB, 128, W_COLS + T)} for b in range(B)]

    trace = bool(int(os.environ.get("KERNEL_TRACE", "0")))
    res = run_bass_kernel_spmd(nc, in_maps, core_ids=list(range(N_CORES)),
                               trace=trace)
    global LAST_RESULTS
    LAST_RESULTS = res

    out = np.empty((B, T, H), dtype=np.float32)
    for b in range(B):
        oT = np.concatenate(
            [blk.astype(np.float32) for blk in res.results[b]["outT"]],
            axis=1)
        l = np.concatenate(
            [blk.astype(np.float32).sum(axis=0)
             for blk in res.results[b]["asum"]])
        out[b] = (oT / l[None, :]).T
    return out
